# revision 2
# baseline (speedup 1.0000x reference)
"""Trainium2 Bass kernel: batched affine warp (bilinear, zero-fill) + bilinear resize.

Full inputs -> full outputs; batch sharded over 8 NeuronCores (4 images each).

v2: hat-weight formulation with fused accumulate on the DVE:
  G(p) = sum_s hat(t(p) - s) * U[A + s],  hat(x) = relu(min(x+1, 1-x))
each term is ONE custom-DVE op: acc_new = relu(min(u, 2-u)) * Src0 + acc_old,
u = scan(ADD, C1, init=C0) (per-partition affine ramp along the free axis).
No separate adds; 3 channel chains interleaved to hide dependent-op latency.
Pass V uses host-side per-column row alignment (kills the beta*k span; KV ~
|al-1|*128+3). Pass H keeps per-tile dynamic column offsets (KH ~
(|c|+|d-1|)*128+3). Resize = two dense PE matmul stages with exact
antialiased-triangle weights (Ty chunks double as Tx chunks).
"""

import sys

sys.path.insert(0, "/opt/trn_rl_repo")

import numpy as np

from concourse import bacc, bass, mybir, tile
from concourse.bass_utils import run_bass_kernel_spmd

N_CORES = 8
B, H, W, C = 32, 512, 512, 3
OUT = 224
SLOTS = B // N_CORES

PB = 128          # rows per block (both passes)
NBLK = 4
FT = 128          # cols per tile
NT = 4
PADP = 112        # horizontal zero padding (pixels) each side of ud rows
UW = W + 2 * PADP
DT = mybir.dt.float32
DTH = mybir.dt.float16
F32 = np.float32
F16 = np.float16
EPS = 1e-3

_cache = {}


def _get_ops():
    if "ops" in _cache:
        return _cache["ops"]
    from concourse.dve_spec import Spec, Src0, Src1, C0, C1, C2, minn, relu, lower, scan
    from concourse.dve_spec import AluOp
    from concourse.dve_ops import DveOp, OPS
    from concourse.dve_table_gen import DveOpSpec
    import concourse.dve_ops as _do

    from concourse.dve_spec import Zero, One, maxx
    ramp = scan(AluOp.ADD, C1, init=C0)  # u(k) = C0 + (k+1)*C1

    def ref_cl(in0, in1, s0, s1, imm2):
        u = s0 + s1 * (1 + np.arange(in0.shape[-1], dtype=np.float32))[None, :]
        w = np.clip(u, 0.0, 1.0)
        return (w * in0 + in1).astype(np.float32)

    def ref_hat(in0, in1, s0, s1, imm2):
        u = s0 + s1 * (1 + np.arange(in0.shape[-1], dtype=np.float32))[None, :]
        w = np.maximum(np.minimum(u, imm2 - u), 0.0)
        return (w * in0 + in1).astype(np.float32)

    def ref_hat0(in0, in1, s0, s1, imm2):
        u = s0 + s1 * (1 + np.arange(in0.shape[-1], dtype=np.float32))[None, :]
        w = np.maximum(np.minimum(u, imm2 - u), 0.0)
        return (w * in0).astype(np.float32)

    ops = []
    for name, body, ref in (
        ("WARP_CLACC", maxx(minn(ramp, One), Zero) * Src0 + Src1, ref_cl),
        ("WARP_HATA", relu(minn(ramp, C2 - ramp)) * Src0 + Src1, ref_hat),
        ("WARP_HAT0", relu(minn(ramp, C2 - ramp)) * Src0, ref_hat0),
    ):
        spec = Spec(body=body, reference=ref)
        shas = {}
        for ver in ("v3", "v4"):
            try:
                uops = lower(spec, ver=ver)
                shas[ver] = DveOpSpec(
                    name=name, opcode=0, uops=uops,
                    rd1_en=(name != "WARP_HAT0")).sha(ver)
            except Exception:
                pass
        op = DveOp(name, spec, subdim=False, uops_sha=shas)
        OPS.append(op)
        _do.CUSTOM_DVE_SPECS[op.name] = op.spec
        _do._SUB_OPCODE_FOR_NAME[op.name] = _do._CUSTOM_DVE_ROW_BASE + len(OPS) - 1
        assert _do._SUB_OPCODE_FOR_NAME[op.name] < 0x20
        ops.append(op)
    _cache["ops"] = tuple(ops)
    return _cache["ops"]


def _resize_matrix(n_in, n_out):
    scale = n_in / n_out
    out = np.zeros((n_out, n_in), dtype=np.float64)
    for i in range(n_out):
        ctr = (i + 0.5) * scale
        for j in range(int(np.floor(ctr - scale)) - 1, int(np.ceil(ctr + scale)) + 2):
            w = max(0.0, 1.0 - abs((j + 0.5 - ctr) / scale))
            if 0 <= j < n_in:
                out[i, j] = w
        s = out[i].sum()
        if s > 0:
            out[i] /= s
    return out


def _host_prep(images, matrices):
    imgs = np.asarray(images, dtype=F32)
    mats = np.asarray(matrices, dtype=np.float64)

    ps128 = np.arange(PB)
    per_img = []
    for i in range(B):
        a, b, e = mats[i, 0]
        c, d, f = mats[i, 1]
        # Cost of a pass pair ~ |al-1| (V span) + |c| + |d-1| (H span); pick
        # the orientation (optionally transposing the image) that minimizes it.
        cost_n = abs((a - c * (b / d)) - 1) + abs(c) + abs(d - 1)
        cost_t = abs((d - b * (c / a)) - 1) + abs(b) + abs(a - 1)
        transposed = cost_t < cost_n
        if transposed:
            a, b, e, c, d, f = d, c, f, b, a, e
            img = np.ascontiguousarray(imgs[i].transpose(1, 0, 2))
        else:
            img = imgs[i]
        beta = b / d
        alpha = a - c * beta
        gamma = e - f * beta
        al1 = alpha - 1.0
        dm1 = d - 1.0

        # V pass: per-column row alignment. t(r,k) = al1*(p0+r) + beta*k + gamma - r0k
        r0k, t0k, ntv = {}, {}, {}
        KV = 0
        for bb in range(NBLK):
            p0 = PB * bb
            vmin = al1 * p0 + min(al1 * (PB - 1), 0.0)
            for t in range(NT):
                ks = np.arange(FT * t, FT * t + FT, dtype=np.float64)
                base = beta * ks + gamma
                r0 = np.floor(vmin + base - EPS).astype(np.int64)
                r0k[(bb, t)] = r0
                t0 = al1 * p0 + base - r0
                t0k[(bb, t)] = t0
                tmax = (t0 + max(al1 * (PB - 1), 0.0)).max()
                ntv[(bb, t)] = int(np.floor(tmax + EPS)) + 2
                KV = max(KV, ntv[(bb, t)])

        # H pass: per-tile offsets. t_h(p,x) = c*(p0+p) + dm1*(x0+x) + f - hlo
        hlo_, nth, KH = {}, {}, 0
        for bb in range(NBLK):
            p0 = PB * bb
            for t in range(NT):
                x0 = FT * t
                vals = [c * p + dm1 * x + f
                        for p in (p0, p0 + PB - 1) for x in (x0, x0 + FT - 1)]
                lo = int(np.floor(min(vals) - EPS))
                hlo_[(bb, t)] = lo
                nth[(bb, t)] = int(np.floor(max(vals) + EPS)) - lo + 2
                KH = max(KH, nth[(bb, t)])

        per_img.append(dict(img=img, c=c, d=d, f=f, alpha=alpha, beta=beta,
                            gamma=gamma, al1=al1, dm1=dm1, transposed=transposed,
                            r0k=r0k, t0k=t0k, hlo=hlo_, ntv=ntv, nth=nth,
                            KV=KV, KH=KH))

    # K-sorted assignment: slot j gets the 8 images of rank [8j, 8j+8), one per
    # core, so per-slot unrolls are sized to that rank band.
    order = sorted(range(B), key=lambda i: per_img[i]["KV"] + per_img[i]["KH"])
    assign = {}
    KVs, KHs, NTVs, NTHs = [], [], [], []
    for j in range(SLOTS):
        band = order[N_CORES * j:N_CORES * (j + 1)]
        for cix, i in enumerate(band):
            assign[(cix, j)] = i
        KVs.append(max(per_img[i]["KV"] for i in band))
        KHs.append(max(per_img[i]["KH"] for i in band))
        # per-tile term counts: max over the band, so the shared program only
        # emits the ops some core actually needs for that tile
        NTVs.append(tuple(
            max(per_img[i]["ntv"][(bb, t)] for i in band)
            for bb in range(NBLK) for t in range(NT)))
        NTHs.append(tuple(
            max(per_img[i]["nth"][(bb, t)] for i in band)
            for bb in range(NBLK) for t in range(NT)))

    TyT = np.ascontiguousarray(_resize_matrix(H, OUT).T.astype(F16))  # [512, 224]

    cols512 = np.arange(H)
    in_maps = []
    for core in range(N_CORES):
        m = {}
        for j in range(SLOTS):
            KV, KH = KVs[j], KHs[j]
            HALO_R = PB + KV
            SLABW = FT + KH
            pi = per_img[assign[(core, j)]]
            img = pi["img"]
            al1, dm1 = pi["al1"], pi["dm1"]
            halo = np.zeros((NBLK * NT, FT, HALO_R * C), dtype=F16)
            c0v = np.full((NBLK * NT, FT, KV), -1e9, dtype=F32)
            c0h = np.full((NBLK * NT, PB, KH), -1e9, dtype=F32)
            hoff = np.zeros((NBLK * NT,), dtype=np.int32)
            for bb in range(NBLK):
                p0 = PB * bb
                for t in range(NT):
                    ti = bb * NT + t
                    r0 = pi["r0k"][(bb, t)]                       # [FT]
                    rows = p0 + r0[:, None] + np.arange(HALO_R)[None]  # [FT, HALO_R]
                    valid = (rows >= 0) & (rows < H)
                    blk = img[np.clip(rows, 0, H - 1),
                              (FT * t + np.arange(FT))[:, None], :]
                    blk = blk * valid[:, :, None]
                    halo[ti] = blk.reshape(FT, HALO_R * C)
                    t0 = pi["t0k"][(bb, t)]                       # [FT]
                    nterm = int(np.floor((t0 + max(al1 * (PB - 1), 0.0)).max()
                                         + EPS)) + 2
                    sv = np.arange(nterm)
                    c0v[ti, :, :nterm] = (t0[:, None] - sv[None, :]
                                          + 1.0 - al1).astype(F32)
                    hlo = pi["hlo"][(bb, t)]
                    off = (PADP + FT * t + hlo) * C
                    assert 0 <= off and off + SLABW * C <= UW * C, (off, SLABW)
                    hoff[ti] = off
                    vals_max = (pi["c"] * (p0 + PB - 1) if pi["c"] > 0
                                else pi["c"] * p0)
                    c0hp = (pi["c"] * (p0 + ps128) + dm1 * (FT * t)
                            + pi["f"] - hlo)
                    ntermh = int(np.floor((c0hp + max(dm1 * (FT - 1), 0.0)).max()
                                          + EPS)) + 2
                    sh = np.arange(ntermh)
                    c0h[ti, :, :ntermh] = (c0hp[:, None] - sh[None, :]
                                           + 1.0 - dm1).astype(F32)
            m[f"halo{j}"] = halo
            m[f"c0v{j}"] = c0v
            m[f"c0h{j}"] = c0h
            m[f"hoff{j}"] = hoff.reshape(1, -1)
            m[f"beta{j}"] = np.full((128, 1), al1, dtype=F32)
            m[f"dm1{j}"] = np.full((128, 1), dm1, dtype=F32)
        m["tyT"] = TyT
        m["ident"] = np.eye(128, dtype=F16)
        in_maps.append(m)

    return in_maps, dict(KVs=tuple(KVs), KHs=tuple(KHs), NTVs=tuple(NTVs),
                         NTHs=tuple(NTHs), per_img=per_img, assign=assign)


def _build_from_meta(meta):
    return _build_program(meta["KVs"], meta["KHs"], meta["NTVs"], meta["NTHs"])


def _build_program(KVs, KHs, NTVs=None, NTHs=None):
    if NTVs is None:
        NTVs = tuple(tuple(KVs[j] for _ in range(NBLK * NT)) for j in range(SLOTS))
    if NTHs is None:
        NTHs = tuple(tuple(KHs[j] for _ in range(NBLK * NT)) for j in range(SLOTS))
    key = (tuple(KVs), tuple(KHs), NTVs, NTHs)
    if key in _cache:
        return _cache[key]
    OPS3 = _get_ops()

    nc = bacc.Bacc("TRN2", target_bir_lowering=False, debug=False,
                   num_devices=N_CORES)

    halos, c0vs, c0hs, hoffs, betas, dm1s, outs, uds = [], [], [], [], [], [], [], []
    for j in range(SLOTS):
        KV, KH = KVs[j], KHs[j]
        HALO_R = MVs[j] * (2 * (PB // MVs[j]) + KV - 1)
        halos.append(nc.dram_tensor(f"halo{j}", [NBLK * NT, FT, HALO_R * C], DTH,
                                    kind="ExternalInput").ap())
        c0vs.append(nc.dram_tensor(f"c0v{j}", [NBLK * NT, FT, KV], DT,
                                   kind="ExternalInput").ap())
        c0hs.append(nc.dram_tensor(f"c0h{j}", [NBLK * NT, PB, KH], DT,
                                   kind="ExternalInput").ap())
        hoffs.append(nc.dram_tensor(
            f"hoff{j}", [1, NBLK * NT * GRs[j] * MHs[j]], mybir.dt.int32,
            kind="ExternalInput").ap())
        betas.append(nc.dram_tensor(f"beta{j}", [128, 1], DT, kind="ExternalInput").ap())
        dm1s.append(nc.dram_tensor(f"dm1{j}", [128, 1], DT, kind="ExternalInput").ap())
        outs.append(nc.dram_tensor(f"out{j}", [C, OUT, OUT], DT, kind="ExternalOutput").ap())
        uds.append(nc.dram_tensor(f"ud{j}", [H, UW * C], DTH).ap())
    tyT = nc.dram_tensor("tyT", [H, OUT], DTH, kind="ExternalInput").ap()
    identd = nc.dram_tensor("ident", [128, 128], DTH, kind="ExternalInput").ap()

    with tile.TileContext(nc) as tc:
        with (
            tc.tile_pool(name="const", bufs=1) as constp,
            tc.tile_pool(name="halo", bufs=3) as halop,
            tc.tile_pool(name="tabs", bufs=4) as tabp,
            tc.tile_pool(name="acc", bufs=4) as accp,
            tc.tile_pool(name="slab", bufs=3) as slabp,
            tc.tile_pool(name="wout", bufs=2) as woutp,
            tc.tile_pool(name="ublk", bufs=2) as ublkp,
            tc.tile_pool(name="rsz", bufs=2) as rszp,
            tc.tile_pool(name="c1T", bufs=3) as c1Tp,
            tc.tile_pool(name="psum1", bufs=2, space="PSUM") as psump,
            tc.tile_pool(name="psumT", bufs=2, space="PSUM") as psumTp,
            tc.tile_pool(name="psumD", bufs=1, space="PSUM") as psumDp,
        ):
            ty_tiles = []
            for kt in range(4):
                t_ = constp.tile([128, OUT], DTH, tag=f"ty{kt}")
                nc.sync.dma_start(t_[:], tyT[128 * kt:128 * kt + 128, :])
                ty_tiles.append(t_)
            ident = constp.tile([128, 128], DTH, tag="ident")
            nc.sync.dma_start(ident[:], identd[:])
            zpad = constp.tile([PB, PADP * C], DTH, tag="zpad")
            nc.vector.memset(zpad[:], 0.0)
            dsem = nc.alloc_semaphore("dsem")
            dsem_cnt = 0

            for j in range(SLOTS):
                KV, KH = KVs[j], KHs[j]
                HALO_R = PB + KV
                SLABW = FT + KH
                beta_t = constp.tile([128, 1], DT, tag=f"beta{j}")
                nc.sync.dma_start(beta_t[:], betas[j][:])
                dm1_t = constp.tile([128, 1], DT, tag=f"dm1{j}")
                nc.sync.dma_start(dm1_t[:], dm1s[j][:])
                hoff_t = constp.tile([1, NBLK * NT * gr * mh], mybir.dt.int32,
                                     tag=f"hoff{j}")
                nc.sync.dma_start(hoff_t[:], hoffs[j][:])

                wouts = []
                for bb in range(NBLK):
                    p0 = PB * bb
                    ublk = ublkp.tile([PB, UW * C], DTH, tag="ublk")
                    for t in range(NT):
                        ti = bb * NT + t
                        halo_t = halop.tile([FT, HALO_R * C], DTH, tag="halo")
                        nc.gpsimd.dma_start(halo_t[:], halos[j][ti])
                        c0v_t = tabp.tile([FT, KV], DT, tag="c0v")
                        nc.gpsimd.dma_start(c0v_t[:], c0vs[j][ti])
                        accA = accp.tile([FT, PB * C], DT, tag="accA")
                        accB = accp.tile([FT, PB * C], DT, tag="accB")
                        h3 = halo_t[:].rearrange("p (r c) -> p r c", c=C)
                        a3 = accA[:].rearrange("p (r c) -> p r c", c=C)
                        b3 = accB[:].rearrange("p (r c) -> p r c", c=C)
                        NV = NTVs[j][ti]
                        for s in range(NV):
                            dst, src = (a3, b3) if s % 2 == 0 else (b3, a3)
                            for ch in range(C):
                                if s == 0:
                                    nc.vector._custom_dve(
                                        H0,
                                        out=dst[:, :, ch],
                                        in0=h3[:, s:s + PB, ch],
                                        s0=c0v_t[:, s:s + 1],
                                        s1=beta_t[0:FT, :],
                                        imm2=2.0,
                                    )
                                else:
                                    nc.vector._custom_dve(
                                        HACC,
                                        out=dst[:, :, ch],
                                        in0=h3[:, s:s + PB, ch],
                                        in1=src[:, :, ch],
                                        s0=c0v_t[:, s:s + 1],
                                        s1=beta_t[0:FT, :],
                                        imm2=2.0,
                                    )
                        fin = a3 if (NV - 1) % 2 == 0 else b3
                        u3 = ublk[:].rearrange("p (k c) -> p k c", c=C)
                        for ch in range(C):
                            tp = psumTp.tile([PB, FT], DTH, tag="vtp")
                            nc.tensor.transpose(tp[:], fin[:, :, ch], ident[:])
                            nc.scalar.copy(
                                u3[:, PADP + FT * t:PADP + FT * (t + 1), ch], tp[:])
                    nc.scalar.dma_start(
                        uds[j][p0:p0 + PB, PADP * C:(PADP + W) * C],
                        ublk[:, PADP * C:(PADP + W) * C])
                    nc.scalar.dma_start(uds[j][p0:p0 + PB, 0:PADP * C], zpad[:])
                    nc.scalar.dma_start(uds[j][p0:p0 + PB, (PADP + W) * C:], zpad[:])

                for bb in range(NBLK):
                    p0 = PB * bb
                    wout = woutp.tile([PB, W * C], DTH, tag=f"wout{bb}")
                    w3 = wout[:].rearrange("p (k c) -> p k c", c=C)
                    for t in range(NT):
                        ti = bb * NT + t
                        c0h_t = tabp.tile([128, KH], DT, tag="c0h")
                        nc.scalar.dma_start(c0h_t[:], c0hs[j][ti])
                        slab = slabp.tile([PB, SLABW * C], DTH, tag="slab")
                        with tc.tile_critical():
                            for g in range(gr):
                                for h in range(mh):
                                    gi = ti * gr * mh + g * mh + h
                                    dsem_cnt += 16
                                    with nc.sync.register(f"ho{j}_{gi}") as reg:
                                        nc.sync.load(reg, hoff_t[0:1, gi:gi + 1])
                                        off = nc.sync.snap(reg)
                                        nc.sync.dma_start(
                                            slab[g * Pg:(g + 1) * Pg,
                                                 h * SEGH * C:(h + 1) * SEGH * C],
                                            uds[j][p0 + g * Pg:p0 + (g + 1) * Pg][
                                                :, bass.ds(off, SEGH * C)],
                                        ).then_inc(dsem, 16)
                            nc.sync.wait_ge(dsem, dsem_cnt)
                        accA = accp.tile([PB, FT * C], DT, tag="acchA")
                        accB = accp.tile([PB, FT * C], DT, tag="acchB")
                        s3 = slab[:].rearrange("p (k c) -> p k c", c=C)
                        a3 = accA[:].rearrange("p (k c) -> p k c", c=C)
                        b3 = accB[:].rearrange("p (k c) -> p k c", c=C)
                        w3t = w3[:, FT * t:FT * (t + 1), :]
                        NH = NTHs[j][ti]
                        for s in range(NH):
                            last = s == NH - 1
                            dst, src = (a3, b3) if s % 2 == 0 else (b3, a3)
                            if last:
                                dst = w3t
                            for ch in range(C):
                                if s == 0:
                                    nc.vector._custom_dve(
                                        H0,
                                        out=dst[:, :, ch],
                                        in0=s3[:, s:s + FT, ch],
                                        s0=c0h_t[0:PB, s:s + 1],
                                        s1=dm1_t[0:PB, :],
                                        imm2=2.0,
                                    )
                                else:
                                    nc.vector._custom_dve(
                                        HACC,
                                        out=dst[:, :, ch],
                                        in0=s3[:, s:s + FT, ch],
                                        in1=src[:, :, ch],
                                        s0=c0h_t[0:PB, s:s + 1],
                                        s1=dm1_t[0:PB, :],
                                        imm2=2.0,
                                    )
                    wouts.append(wout)

                # ---------- resize ----------
                # stage 1: C1[oy, j] = sum_y Ty[oy, y] * wout[y, j]
                c1sb = [[None] * 4 for _ in range(2)]
                for mm in range(2):
                    for kt in range(4):
                        ps1 = psump.tile([112, FT * C], DT, tag="c1ps")
                        for bb in range(NBLK):
                            nc.tensor.matmul(
                                ps1[:],
                                ty_tiles[bb][:, 112 * mm:112 * (mm + 1)],
                                wouts[bb][:, FT * C * kt:FT * C * (kt + 1)],
                                start=(bb == 0),
                                stop=(bb == NBLK - 1),
                            )
                        sb = rszp.tile([112, FT * C], DTH, tag=f"c1sb_{mm}_{kt}")
                        nc.scalar.copy(sb[:], ps1[:])
                        c1sb[mm][kt] = sb

                # stage 2: per channel transpose then X-contraction
                for ch in range(C):
                    for mm in range(2):
                        dps = psumDp.tile([112, OUT], DT, tag="dps")
                        for kt in range(4):
                            sb = c1sb[mm][kt]
                            s3 = sb[:].rearrange("p (k c) -> p k c", c=C)
                            tps = psumTp.tile([128, 112], DTH, tag="tps")
                            nc.tensor.transpose(tps[:], s3[:, :, ch], ident[0:112, 0:112])
                            tsb = c1Tp.tile([128, 112], DTH, tag="tsb")
                            nc.scalar.copy(tsb[:], tps[:])
                            nc.tensor.matmul(
                                dps[:], tsb[:], ty_tiles[kt][:],
                                start=(kt == 0), stop=(kt == 3),
                            )
                        dsb = rszp.tile([112, OUT], DT, tag="dsb")
                        nc.scalar.copy(dsb[:], dps[:])
                        nc.sync.dma_start(outs[j][ch, 112 * mm:112 * (mm + 1), :], dsb[:])

    nc.compile()
    _cache[key] = nc
    return nc


def kernel(images, matrices):
    in_maps, meta = _host_prep(images, matrices)
    nc = _build_from_meta(meta)
    res = run_bass_kernel_spmd(nc, in_maps, list(range(N_CORES)))
    out = np.zeros((B, OUT, OUT, C), dtype=F32)
    for core in range(N_CORES):
        for j in range(SLOTS):
            i = meta["assign"][(core, j)]
            o = res.results[core][f"out{j}"].reshape(C, OUT, OUT).transpose(1, 2, 0)
            if meta["per_img"][i]["transposed"]:
                o = o.transpose(1, 0, 2)
            out[i] = o
    return out


if __name__ == "__main__":
    rng = np.random.default_rng(0)
    imgs = rng.random((B, H, W, C), dtype=np.float32)
    mats = (np.eye(2, 3)[None] + 0.05 * rng.standard_normal((B, 2, 3))).astype(np.float32)
    o = kernel(images=imgs, matrices=mats)
    print("kernel out", o.shape, o.dtype, float(o.min()), float(o.max()))
